# revision 1
# baseline (speedup 1.0000x reference)
"""Multi-head causal self-attention (B=2, T=2048, D=1024, H=16) on 8 trn2 cores.

Sharding: data-parallel over batch (cores 0-3 -> batch 0, 4-7 -> batch 1),
tensor-parallel over heads within each 4-core group (4 heads per core).
Wq/Wk/Wv column-sharded, Wo row-sharded; each core emits its partial output
projection and the host sums the 4 partials per batch (TP unshard).

Per-core pipeline (bf16 matmul operands, fp32 PSUM accumulation):
  x [2048,1024] -> bf16 -> PE transpose -> xT [1024,2048]
  qT/kT = W_slice @ x.T   (heads on partitions, 2-head pairs stacked 128-wide)
  v     = x @ Wv_slice.T  (natural layout, +ones column for softmax denom)
  per (512-query block, head-pair): stream 128-key tiles:
     scoresT pair -> one 2-bank psum tile [128k, 2head*512q] (row-packed K=64 matmuls)
     expT = exp(0.125*scoresT)  (single ACT call over both heads, psum->sbuf bf16)
     causal mask on diagonal tiles (gpsimd affine_select, fill 0)
     out_augT += v_aug.T @ expT (psum [65,512]: rows 0-63 att, row 64 denom)
  normalize per (qb,hp): denom rows lane-packed via sbuf DMA for parallel
  reciprocal, partition-broadcast via DMA, single DVE mul psum->attT (bf16)
  out_partial(qb) = attT.T @ WoT interleaved with next query block's attention
"""

import sys

for _p in ("/opt/trn_rl_repo", "/root/.axon_site/_ro/trn_rl_repo"):
    if _p not in sys.path:
        sys.path.append(_p)

import ml_dtypes
import numpy as np

import concourse.bass as bass
import concourse.mybir as mybir
import concourse.tile as tile
from concourse import bacc
from concourse.bass_utils import run_bass_kernel_spmd
from concourse.masks import make_identity

F32 = mybir.dt.float32
BF16 = mybir.dt.bfloat16

B, T, D = 2, 2048, 1024
H, DH = 16, 64
HPC = 4          # heads per core
FPC = HPC * DH   # feature dims per core (256)
NKT = T // 128   # 16 key tiles / token tiles
NQB = T // 512   # 4 query blocks
VW = DH + 1      # v width incl ones column (65)

_CACHE = {}


def _build():
    nc = bacc.Bacc("TRN2", target_bir_lowering=False, debug=False, num_devices=8)

    xt_d = nc.dram_tensor("xt", [D, T], BF16, kind="ExternalInput").ap()
    wq_d = nc.dram_tensor("wq_t", [128, 8 * FPC], BF16, kind="ExternalInput").ap()
    wk_d = nc.dram_tensor("wk_t", [128, 8 * FPC], BF16, kind="ExternalInput").ap()
    wv_d = nc.dram_tensor("wv_t", [128, 8 * FPC], BF16, kind="ExternalInput").ap()
    wo_d = nc.dram_tensor("wo_t", [128, 2 * D], BF16, kind="ExternalInput").ap()
    onesb_d = nc.dram_tensor("ones_b", [128, 64], BF16, kind="ExternalInput").ap()
    masks_d = nc.dram_tensor("masks", [128, 4 * 1024], BF16, kind="ExternalInput").ap()
    out_d = nc.dram_tensor("po", [T, D], BF16, kind="ExternalOutput").ap()
    rscr_d = nc.dram_tensor("rscr", [8, 1024], F32).ap()

    with tile.TileContext(nc) as tc:
        with (
            tc.tile_pool(name="wp", bufs=1) as wp,
            tc.tile_pool(name="qk", bufs=1) as qk,
            tc.tile_pool(name="vp", bufs=1) as vp,
            tc.tile_pool(name="at", bufs=1) as at,
        ):
            masks_sb = wp.tile([128, 4 * 1024], BF16)
            nc.sync.dma_start(masks_sb[:], masks_d)
            qT_sb = qk.tile([128, 2 * T], BF16)   # head-pair hp at cols hp*T
            kT_sb = qk.tile([128, 2 * T], BF16)
            v_sb = vp.tile([128, NKT * HPC * VW], BF16)
            attT_sb = at.tile([128, 2 * T], BF16)

            # ---- phase 1+2: transpose x, projections ----
            with (
                tc.tile_pool(name="xt", bufs=1) as xtp,
                tc.tile_pool(name="xn", bufs=3) as xnp,
                tc.tile_pool(name="ps12", bufs=1, space="PSUM") as ps12,
            ):
                # x^T chunks (host-pretransposed): contiguous copies, round-robin queues
                wq_sb = wp.tile([128, 8 * FPC], BF16)
                nc.sync.dma_start(wq_sb[:], wq_d)
                wk_sb = wp.tile([128, 8 * FPC], BF16)
                nc.sync.dma_start(wk_sb[:], wk_d)
                wv_sb = wp.tile([128, 8 * FPC], BF16)
                nc.sync.dma_start(wv_sb[:], wv_d)
                xT = []
                for kc in range(8):
                    xT_c = xtp.tile([128, T], BF16, tag=f"xT{kc}")
                    xT.append(xT_c)
                    nc.sync.dma_start(xT_c[:], xt_d[kc * 128 : (kc + 1) * 128, :])
                wo_sb = wp.tile([128, 2 * D], BF16)
                nc.sync.dma_start(wo_sb[:], wo_d)
                nc.sync.dma_start(
                    v_sb[:].rearrange("p (a b) -> p a b", b=VW)[:, :, 64],
                    onesb_d[:, 0 : NKT * HPC],
                )

                # qT / kT projections: [feat(128=2 heads), tok] blocks
                for tb in range(NQB):
                    q_ps = ps12.tile([128, 512], F32, tag="proj", bufs=2)
                    k_ps = ps12.tile([128, 512], F32, tag="proj", bufs=2)
                    for kc in range(8):
                        nc.tensor.matmul(
                        q_ps[:],
                        wq_sb[:, kc * FPC + 0 * 128 : kc * FPC + (0 + 1) * 128],
                        xT[kc][:, tb * 512 : (tb + 1) * 512],
                        start=(kc == 0), stop=(kc == 7),
                        )
                    for kc in range(8):
                        nc.tensor.matmul(
                        k_ps[:],
                        wk_sb[:, kc * FPC + 0 * 128 : kc * FPC + (0 + 1) * 128],
                        xT[kc][:, tb * 512 : (tb + 1) * 512],
                        start=(kc == 0), stop=(kc == 7),
                        )
                    nc.vector.tensor_copy(
                        qT_sb[:, 0 * T + tb * 512 : 0 * T + (tb + 1) * 512], q_ps[:]
                    )
                    nc.vector.tensor_copy(
                        kT_sb[:, 0 * T + tb * 512 : 0 * T + (tb + 1) * 512], k_ps[:]
                    )

                # v projection: natural [tok, feat] tiles
                for tt in range(NKT):
                    v_ps = ps12.tile([128, FPC], F32, tag="vproj", bufs=2)
                    for kc in range(8):
                        nc.tensor.matmul(
                            v_ps[:],
                            xT[kc][:, tt * 128 : (tt + 1) * 128],
                            wv_sb[:, kc * FPC : (kc + 1) * FPC],
                            start=(kc == 0), stop=(kc == 7),
                        )
                    nc.vector.tensor_copy(
                        v_sb[:].rearrange("p (a b) -> p a b", b=VW)[
                            :, tt * HPC : (tt + 1) * HPC, 0:DH
                        ],
                        v_ps[:].rearrange("p (a b) -> p a b", b=DH),
                    )

                # qT / kT projections for head pair 1
                for tb in range(NQB):
                    q_ps = ps12.tile([128, 512], F32, tag="proj", bufs=2)
                    k_ps = ps12.tile([128, 512], F32, tag="proj", bufs=2)
                    for kc in range(8):
                        nc.tensor.matmul(
                        q_ps[:],
                        wq_sb[:, kc * FPC + 1 * 128 : kc * FPC + (1 + 1) * 128],
                        xT[kc][:, tb * 512 : (tb + 1) * 512],
                        start=(kc == 0), stop=(kc == 7),
                        )
                    for kc in range(8):
                        nc.tensor.matmul(
                        k_ps[:],
                        wk_sb[:, kc * FPC + 1 * 128 : kc * FPC + (1 + 1) * 128],
                        xT[kc][:, tb * 512 : (tb + 1) * 512],
                        start=(kc == 0), stop=(kc == 7),
                        )
                    nc.vector.tensor_copy(
                        qT_sb[:, 1 * T + tb * 512 : 1 * T + (tb + 1) * 512], q_ps[:]
                    )
                    nc.vector.tensor_copy(
                        kT_sb[:, 1 * T + tb * 512 : 1 * T + (tb + 1) * 512], k_ps[:]
                    )

            # ---- phase 3            # ---- phase 3: attention + per-block output projection ----
            with (
                tc.tile_pool(name="ep", bufs=4) as ep,
                tc.tile_pool(name="nr", bufs=2) as nrm,
                tc.tile_pool(name="op", bufs=3) as op,
                tc.tile_pool(name="ps3", bufs=1, space="PSUM") as ps3,
            ):
                for qb in range(NQB):
                    for hp in range(2):
                        hA, hB = 2 * hp, 2 * hp + 1
                        oA = ps3.tile([VW, 512], F32, tag="oA", bufs=2)
                        oB = ps3.tile([VW, 512], F32, tag="oB", bufs=2)
                        nkt = 4 * (qb + 1)

                        def attv(e, kt, nkt=nkt, oA=oA, oB=oB, hA=hA, hB=hB):
                            nc.tensor.matmul(
                                oA[:],
                                v_sb[:, (kt * HPC + hA) * VW : (kt * HPC + hA + 1) * VW],
                                e[:, 0:512],
                                start=(kt == 0), stop=(kt == nkt - 1),
                            )
                            nc.tensor.matmul(
                                oB[:],
                                v_sb[:, (kt * HPC + hB) * VW : (kt * HPC + hB + 1) * VW],
                                e[:, 512:1024],
                                start=(kt == 0), stop=(kt == nkt - 1),
                            )

                        pend = []
                        for kt in range(nkt):
                            sAB = ps3.tile([128, 1024], F32, tag="sAB", bufs=2)
                            nc.tensor.matmul(
                                sAB[:, 0:512],
                                kT_sb[0:64, hp * T + kt * 128 : hp * T + (kt + 1) * 128],
                                qT_sb[0:64, hp * T + qb * 512 : hp * T + (qb + 1) * 512],
                                start=True, stop=True, tile_position=(0, 0),
                            )
                            nc.tensor.matmul(
                                sAB[:, 512:1024],
                                kT_sb[64:128, hp * T + kt * 128 : hp * T + (kt + 1) * 128],
                                qT_sb[64:128, hp * T + qb * 512 : hp * T + (qb + 1) * 512],
                                start=True, stop=True, tile_position=(64, 0),
                            )
                            eAB = ep.tile([128, 1024], BF16, tag="eAB")
                            nc.scalar.activation(
                                eAB[:], sAB[:], mybir.ActivationFunctionType.Exp,
                                scale=0.125,
                            )
                            r = kt - 4 * qb
                            if r >= 0:  # diagonal tile: mask k > q
                                nc.vector.tensor_mul(
                                    eAB[:], eAB[:],
                                    masks_sb[:, r * 1024 : (r + 1) * 1024],
                                )
                            pend.append((eAB, kt))
                            if len(pend) > 2:
                                attv(*pend.pop(0))
                        for a in pend:
                            attv(*a)
                        # normalize (qb, hp): pack denoms, reciprocal, bcast, mul
                        srows = nrm.tile([1, 1024], F32, tag="srows")
                        nc.vector.tensor_copy(srows[0:1, 0:512], oA[64:65, :])
                        nc.vector.tensor_copy(srows[0:1, 512:1024], oB[64:65, :])
                        packed = nrm.tile([128, 8], F32, tag="packed")
                        nc.sync.dma_start(
                            packed[:],
                            srows[:].rearrange("r (g e) -> r g e", e=8),
                        )
                        rpacked = nrm.tile([128, 8], F32, tag="rpacked")
                        nc.vector.reciprocal(rpacked[:], packed[:])
                        ridx = qb * 2 + hp
                        rrow_d = rscr_d[ridx : ridx + 1, :]
                        nc.sync.dma_start(
                            rrow_d.rearrange("r (g e) -> r g e", e=8),
                            rpacked[:],
                        )
                        for o_ps, prow, off in ((oA, 0, 0), (oB, 64, 512)):
                            bc = nrm.tile([64, 512], F32, tag="bc")
                            nc.sync.dma_start(
                                bc[:],
                                rrow_d[0:1, off : off + 512].partition_broadcast(64),
                            )
                            nc.vector.tensor_mul(
                                attT_sb[
                                    prow : prow + 64,
                                    hp * T + qb * 512 : hp * T + (qb + 1) * 512,
                                ],
                                o_ps[0:64, :],
                                bc[:],
                            )
                    # output projection for this query block's 4 token tiles
                    for t4 in range(4):
                        tt = qb * 4 + t4
                        o_sb = op.tile([128, D], BF16, tag="osb")
                        for nck in range(2):
                            wo_ps = ps3.tile(
                                [128, 512], F32,
                                tag=("oA" if nck == 0 else "oB"), bufs=2,
                            )
                            for hp in range(2):
                                nc.tensor.matmul(
                                    wo_ps[:],
                                    attT_sb[:, hp * T + tt * 128 : hp * T + (tt + 1) * 128],
                                    wo_sb[:, hp * D + nck * 512 : hp * D + (nck + 1) * 512],
                                    start=(hp == 0), stop=(hp == 1),
                                )
                            nc.vector.tensor_copy(
                                o_sb[:, nck * 512 : (nck + 1) * 512], wo_ps[:]
                            )
                        nc.sync.dma_start(out_d[tt * 128 : (tt + 1) * 128, :], o_sb[:])

    nc.compile()
    return nc


def _prepack(w, bf):
    # [c*128, f] -> [128, c*f] (SBUF chunk layout)
    c = w.shape[0] // 128
    return np.ascontiguousarray(
        w.reshape(c, 128, w.shape[1]).transpose(1, 0, 2).reshape(128, -1)
    ).astype(bf)


def _prep_in_maps(x, Wq, Wk, Wv, Wo):
    x = np.asarray(x, dtype=np.float32)
    bf = ml_dtypes.bfloat16
    Wq = np.asarray(Wq, dtype=np.float32)
    Wk = np.asarray(Wk, dtype=np.float32)
    Wv = np.asarray(Wv, dtype=np.float32)
    Wo = np.asarray(Wo, dtype=np.float32)
    ones_b = np.ones((128, 64), dtype=bf)
    ii = np.arange(128)[:, None]
    qq = np.arange(512)[None, :]
    masks = np.concatenate(
        [np.tile((qq >= ii + 128 * r).astype(bf), (1, 2)) for r in range(4)],
        axis=1,
    )
    in_maps = []
    for c in range(8):
        b, g = divmod(c, 4)
        sl = slice(g * FPC, (g + 1) * FPC)
        in_maps.append(
            {
                "xt": np.ascontiguousarray(x[b].T).astype(bf),
                "wq_t": _prepack(Wq[sl, :].T, bf),
                "wk_t": _prepack(Wk[sl, :].T, bf),
                "wv_t": _prepack(Wv[sl, :].T, bf),
                "wo_t": _prepack(Wo[:, sl].T, bf),
                "ones_b": ones_b,
                "masks": masks,
            }
        )
    return in_maps


def _get_nc():
    if "nc" not in _CACHE:
        _CACHE["nc"] = _build()
    return _CACHE["nc"]


def _assemble(results):
    out = np.empty((B, T, D), dtype=np.float32)
    for b in range(B):
        out[b] = (
            results[4 * b]["po"].astype(np.float32)
            + results[4 * b + 1]["po"].astype(np.float32)
            + results[4 * b + 2]["po"].astype(np.float32)
            + results[4 * b + 3]["po"].astype(np.float32)
        )
    return out


def kernel(x, Wq, Wk, Wv, Wo):
    nc = _get_nc()
    in_maps = _prep_in_maps(x, Wq, Wk, Wv, Wo)
    res = run_bass_kernel_spmd(nc, in_maps, core_ids=list(range(8)))
    return _assemble(res.results)


def kernel_with_trace(x, Wq, Wk, Wv, Wo, **kw):
    nc = _get_nc()
    in_maps = _prep_in_maps(x, Wq, Wk, Wv, Wo)
    res = run_bass_kernel_spmd(nc, in_maps, core_ids=list(range(8)), trace=True, **kw)
    return _assemble(res.results), res



# revision 7
# speedup vs baseline: 1.2119x; 1.2119x over previous
"""Multi-head causal self-attention (B=2, T=2048, D=1024, H=16) on 8 trn2 cores.

Sharding: data-parallel over batch (cores 0-3 -> batch 0, 4-7 -> batch 1),
tensor-parallel over heads within each 4-core group (4 heads per core).
Wq/Wk/Wv column-sharded, Wo row-sharded; each core emits its partial output
projection and the host sums the 4 partials per batch (TP unshard).

v2 pipeline: the scalar engine's exp stream is the critical resource
(~1.2us per [128,1024] score tile).  Everything else is scheduled around
keeping it dense from ~7us onward:
  - projections (q/k/v/out) are dripped into the attention steady state
    (PE has ~35% slack vs ACT), instead of a serial projection phase
  - causal mask applied inside the scores matmul accumulation:
    sAB += tri.T @ xmk adds -1024 on masked entries (no DVE/gpsimd in the
    scores->exp->attV path)
  - diagonal tiles skip fully-masked query columns in both the scores
    matmul and the exp (strided ACT access pattern)
  - softmax denominators via a ones-column in v (attV row 64); the
    reciprocal is packed across partitions via sbuf DMA, bounced through
    DRAM for a partition-broadcast, all off the critical path
  - attV psum accumulators are copied to SBUF immediately so the 2 psum
    banks recycle; PSUM budget: scores 2x2 + oA/oB 2 + proj 2 = 8 banks
"""

import sys
from collections import deque

for _p in ("/opt/trn_rl_repo", "/root/.axon_site/_ro/trn_rl_repo"):
    if _p not in sys.path:
        sys.path.append(_p)

import ml_dtypes
import numpy as np

import concourse.bass as bass
import concourse.mybir as mybir
import concourse.tile as tile
from concourse import bacc
from concourse.bass_utils import run_bass_kernel_spmd

F32 = mybir.dt.float32
BF16 = mybir.dt.bfloat16

DIAG_SKIP = False  # skip fully-masked query columns of diagonal tiles

B, T, D = 2, 2048, 1024
H, DH = 16, 64
HPC = 4          # heads per core
FPC = HPC * DH   # feature dims per core (256)
NKT = T // 128   # 16 key tiles / token tiles
NQB = T // 512   # 4 query blocks
VW = DH + 1      # v width incl ones column (65)

_CACHE = {}


def _build():
    nc = bacc.Bacc("TRN2", target_bir_lowering=False, debug=False, num_devices=8)

    xt_d = nc.dram_tensor("xt", [D, T], BF16, kind="ExternalInput").ap()
    wq_d = nc.dram_tensor("wq_p", [128, 2 * D], BF16, kind="ExternalInput").ap()
    wk_d = nc.dram_tensor("wk_p", [128, 2 * D], BF16, kind="ExternalInput").ap()
    wv_d = nc.dram_tensor("wv_t", [128, 8 * FPC], BF16, kind="ExternalInput").ap()
    wo_d = nc.dram_tensor("wo_t", [128, 2 * D], BF16, kind="ExternalInput").ap()
    onesb_d = nc.dram_tensor("ones_b", [128, 64], BF16, kind="ExternalInput").ap()
    tri_d = nc.dram_tensor("tri", [128, 128], BF16, kind="ExternalInput").ap()
    xmk_d = nc.dram_tensor("xmk", [128, 4 * 512], BF16, kind="ExternalInput").ap()
    out_d = nc.dram_tensor("po", [T, D], BF16, kind="ExternalOutput").ap()
    rscr_d = nc.dram_tensor("rscr", [8, 1024], F32).ap()

    with tile.TileContext(nc) as tc:
        with (
            tc.tile_pool(name="wp", bufs=1) as wp,
            tc.tile_pool(name="xp", bufs=1) as xp,
            tc.tile_pool(name="qk", bufs=1) as qk,
            tc.tile_pool(name="vp", bufs=1) as vp,
            tc.tile_pool(name="at", bufs=1) as at,
            tc.tile_pool(name="ep", bufs=1) as ep,
            tc.tile_pool(name="ob", bufs=1) as ob,
            tc.tile_pool(name="nr", bufs=2) as nr,
            tc.tile_pool(name="op", bufs=3) as op,
            tc.tile_pool(name="pss", bufs=1, space="PSUM") as pss,
            tc.tile_pool(name="pso", bufs=1, space="PSUM") as pso,
            tc.tile_pool(name="psp", bufs=1, space="PSUM") as psp,
        ):
            # ---- persistent SBUF tiles ----
            wq_sb = wp.tile([128, 2 * D], BF16)
            wk_sb = wp.tile([128, 2 * D], BF16)
            wv_sb = wp.tile([128, 8 * FPC], BF16)
            wo_sb = wp.tile([128, 2 * D], BF16)
            tri_sb = wp.tile([128, 128], BF16)
            xmk_sb = wp.tile([128, 4 * 512], BF16)
            xT = [xp.tile([128, T], BF16, tag=f"xT{kc}", name=f"xT{kc}") for kc in range(8)]
            qT_sb = qk.tile([128, 2 * T], BF16)   # pair hp at cols hp*T
            kT_sb = qk.tile([128, 2 * T], BF16)
            v_sb = vp.tile([128, NKT * HPC * VW], BF16)
            attT_sb = at.tile([128, 2 * T], BF16)

            # warm the ACT exp table during the input DMAs
            warm_a = nr.tile([1, 8], F32, tag="warm", bufs=1)
            warm_b = nr.tile([1, 8], F32, tag="warm2", bufs=1)
            nc.vector.memset(warm_a[:], 0.0)
            nc.scalar.activation(warm_b[:], warm_a[:], mybir.ActivationFunctionType.Exp)

            # ---- input DMAs, priority order ----
            nc.sync.dma_start(wq_sb[:, 0:D], wq_d[:, 0:D])
            nc.sync.dma_start(wk_sb[:, 0:D], wk_d[:, 0:D])
            for kc in range(8):
                nc.sync.dma_start(xT[kc][:, 0:512], xt_d[kc * 128 : (kc + 1) * 128, 0:512])
            nc.sync.dma_start(wv_sb[:], wv_d)
            nc.sync.dma_start(tri_sb[:], tri_d)
            nc.sync.dma_start(xmk_sb[:], xmk_d)
            nc.sync.dma_start(
                v_sb[:].rearrange("p (a b) -> p a b", b=VW)[:, :, 64],
                onesb_d[:, 0 : NKT * HPC],
            )
            nc.sync.dma_start(wq_sb[:, D : 2 * D], wq_d[:, D : 2 * D])
            nc.sync.dma_start(wk_sb[:, D : 2 * D], wk_d[:, D : 2 * D])
            for kc in range(8):
                nc.sync.dma_start(
                    xT[kc][:, 512:T], xt_d[kc * 128 : (kc + 1) * 128, 512:T]
                )
            nc.sync.dma_start(wo_sb[:], wo_d)

            # ---- emission helpers ----
            emitted = set()
            backlog = deque()
            pend = deque()

            def emit_qk(hp, tb):
                for w_sb, dst in ((wq_sb, qT_sb), (wk_sb, kT_sb)):
                    ps = psp.tile([128, 512], F32, tag="proj", bufs=2, name="qk_ps")
                    for kc in range(8):
                        nc.tensor.matmul(
                            ps[:],
                            w_sb[:, hp * D + kc * 128 : hp * D + (kc + 1) * 128],
                            xT[kc][:, tb * 512 : (tb + 1) * 512],
                            start=(kc == 0), stop=(kc == 7),
                        )
                    nc.vector.tensor_copy(
                        dst[:, hp * T + tb * 512 : hp * T + (tb + 1) * 512], ps[:]
                    )

            def do_qk(hp, tb):
                key = ("qk", hp, tb)
                if key in emitted:
                    return
                emitted.add(key)
                emit_qk(hp, tb)

            def emit_v(tt):
                ps = psp.tile([128, 512], F32, tag="proj", bufs=2, name="v_ps")
                for kc in range(8):
                    nc.tensor.matmul(
                        ps[:, 0:FPC],
                        xT[kc][:, tt * 128 : (tt + 1) * 128],
                        wv_sb[:, kc * FPC : (kc + 1) * FPC],
                        start=(kc == 0), stop=(kc == 7),
                    )
                nc.vector.tensor_copy(
                    v_sb[:].rearrange("p (a b) -> p a b", b=VW)[
                        :, tt * HPC : (tt + 1) * HPC, 0:DH
                    ],
                    ps[:, 0:FPC].rearrange("p (a b) -> p a b", b=DH),
                )

            def do_v(tt):
                key = ("v", tt)
                if key in emitted:
                    return
                emitted.add(key)
                emit_v(tt)

            def emit_outproj_tt(qb, t4):
                tt = qb * 4 + t4
                o_sb = op.tile([128, D], BF16, tag="osb", name="o_sb")
                for nck in range(2):
                    wo_ps = psp.tile([128, 512], F32, tag="proj", bufs=2, name="wo_ps")
                    for hp in range(2):
                        nc.tensor.matmul(
                            wo_ps[:],
                            attT_sb[:, hp * T + tt * 128 : hp * T + (tt + 1) * 128],
                            wo_sb[:, hp * D + nck * 512 : hp * D + (nck + 1) * 512],
                            start=(hp == 0), stop=(hp == 1),
                        )
                    nc.vector.tensor_copy(
                        o_sb[:, nck * 512 : (nck + 1) * 512], wo_ps[:]
                    )
                nc.sync.dma_start(out_d[tt * 128 : (tt + 1) * 128, :], o_sb[:])

            def emit_scores(qb, hp, kt):
                sAB = pss.tile([128, 1024], F32, tag="sAB", bufs=2, name="sAB")
                r = kt - 4 * qb
                qs = 128 * r if (r > 0 and DIAG_SKIP) else 0
                for h, tp in ((0, (0, 0)), (1, (64, 0))):
                    nc.tensor.matmul(
                        sAB[:, h * 512 + qs : (h + 1) * 512],
                        kT_sb[h * 64 : (h + 1) * 64, hp * T + kt * 128 : hp * T + (kt + 1) * 128],
                        qT_sb[h * 64 : (h + 1) * 64, hp * T + qb * 512 + qs : hp * T + (qb + 1) * 512],
                        start=True, stop=(r < 0), tile_position=tp,
                    )
                    if r >= 0:  # causal mask add: -1024 on masked entries
                        if DIAG_SKIP:
                            nc.tensor.matmul(
                                sAB[:, h * 512 + qs : h * 512 + qs + 128],
                                tri_sb[:],
                                xmk_sb[:, 0:128],
                                start=False, stop=True,
                            )
                        else:
                            nc.tensor.matmul(
                                sAB[:, h * 512 : (h + 1) * 512],
                                tri_sb[:],
                                xmk_sb[:, r * 512 : (r + 1) * 512],
                                start=False, stop=True,
                            )
                return sAB, qs

            def emit_act(sAB, qs):
                eAB = ep.tile([128, 1024], BF16, tag="eAB", bufs=4, name="eAB")
                if qs == 0:
                    nc.scalar.activation(
                        eAB[:], sAB[:], mybir.ActivationFunctionType.Exp, scale=0.125
                    )
                else:
                    iv = sAB[:].rearrange("p (h q) -> p h q", h=2)[:, :, qs:512]
                    ov = eAB[:].rearrange("p (h q) -> p h q", h=2)[:, :, qs:512]
                    nc.scalar.activation(
                        ov, iv, mybir.ActivationFunctionType.Exp, scale=0.125
                    )
                return eAB

            def emit_attv(qb, hp, kt, eAB, qs, oA, oB):
                nkt = 4 * (qb + 1)
                for h, o_ps in ((0, oA), (1, oB)):
                    nc.tensor.matmul(
                        o_ps[:, qs:512],
                        v_sb[:, (kt * HPC + 2 * hp + h) * VW : (kt * HPC + 2 * hp + h + 1) * VW],
                        eAB[:, h * 512 + qs : (h + 1) * 512],
                        start=(kt == 0), stop=(kt == nkt - 1),
                    )

            def emit_norm(ui, qb, hp, oA, oB):
                oAs = ob.tile([VW, 512], F32, tag="oAs", bufs=2, name="oAs")
                oBs = ob.tile([VW, 512], F32, tag="oBs", bufs=2, name="oBs")
                nc.vector.tensor_copy(oAs[:], oA[:])
                nc.vector.tensor_copy(oBs[:], oB[:])
                packed = nr.tile([128, 8], F32, tag="packed", name="packed")
                nc.sync.dma_start(
                    packed[:, 0:4], oAs[64:65, :].rearrange("r (g e) -> r g e", e=4)
                )
                nc.sync.dma_start(
                    packed[:, 4:8], oBs[64:65, :].rearrange("r (g e) -> r g e", e=4)
                )
                rpacked = nr.tile([128, 8], F32, tag="rpacked", name="rpacked")
                nc.vector.reciprocal(rpacked[:], packed[:])
                rrow_d = rscr_d[ui : ui + 1, :]
                nc.sync.dma_start(
                    rrow_d[:, 0:512].rearrange("r (g e) -> r g e", e=4), rpacked[:, 0:4]
                )
                nc.sync.dma_start(
                    rrow_d[:, 512:1024].rearrange("r (g e) -> r g e", e=4), rpacked[:, 4:8]
                )
                for o_s, prow, off in ((oAs, 0, 0), (oBs, 64, 512)):
                    bc = nr.tile([64, 512], F32, tag="bc", name="bc")
                    nc.sync.dma_start(
                        bc[:], rrow_d[0:1, off : off + 512].partition_broadcast(64)
                    )
                    nc.vector.tensor_mul(
                        attT_sb[prow : prow + 64, hp * T + qb * 512 : hp * T + (qb + 1) * 512],
                        o_s[0:64, :],
                        bc[:],
                    )

            def pop_attv():
                qb_, hp_, kt_, ui_, eAB_, qs_, oA_, oB_ = pend.popleft()
                do_v(kt_)
                emit_attv(qb_, hp_, kt_, eAB_, qs_, oA_, oB_)
                if kt_ == 4 * (qb_ + 1) - 1:
                    emit_norm(ui_, qb_, hp_, oA_, oB_)
                    if hp_ == 1:
                        for t4 in range(4):
                            backlog.append(
                                lambda qb=qb_, t4=t4: emit_outproj_tt(qb, t4)
                            )

            def drip(n):
                for _ in range(n):
                    if backlog:
                        backlog.popleft()()

            # work pushed at each unit start, consumed by later units
            unit_pushes = {
                0: [lambda: do_qk(1, 0)],
                1: [lambda: do_qk(0, 1)] + [
                    (lambda tt=tt: do_v(tt)) for tt in range(4, 8)
                ],
                2: [lambda: do_qk(1, 1)],
                3: [lambda: do_qk(0, 2)] + [
                    (lambda tt=tt: do_v(tt)) for tt in range(8, 12)
                ],
                4: [lambda: do_qk(1, 2)],
                5: [lambda: do_qk(0, 3)] + [
                    (lambda tt=tt: do_v(tt)) for tt in range(12, 16)
                ],
                6: [lambda: do_qk(1, 3)],
                7: [],
            }

            # ---- main pipeline ----
            do_qk(0, 0)
            ui = 0
            for qb in range(NQB):
                for hp in range(2):
                    do_qk(hp, qb)
                    backlog.extend(unit_pushes[ui])
                    oA = pso.tile([VW, 512], F32, tag="oA", bufs=1, name="oA")
                    oB = pso.tile([VW, 512], F32, tag="oB", bufs=1, name="oB")
                    nkt = 4 * (qb + 1)
                    for kt in range(nkt):
                        do_v(kt)  # ensure v ready 2 steps before its attv
                        sAB, qs = emit_scores(qb, hp, kt)
                        eAB = emit_act(sAB, qs)
                        pend.append((qb, hp, kt, ui, eAB, qs, oA, oB))
                        if len(pend) > 2:
                            pop_attv()
                        steps_left = nkt - kt
                        n = 1 if backlog else 0
                        if len(backlog) > steps_left:
                            n = 2
                        drip(n)
                    ui += 1
            while pend:
                pop_attv()
            drip(len(backlog))

    nc.compile()
    return nc


def _prepack(w, bf):
    # [c*128, f] -> [128, c*f] (SBUF chunk layout)
    c = w.shape[0] // 128
    return np.ascontiguousarray(
        w.reshape(c, 128, w.shape[1]).transpose(1, 0, 2).reshape(128, -1)
    ).astype(bf)


def _prepack_pair(w, bf):
    # w: [1024, 256] (d_model x pair features) -> [128, 2*1024] pair-major:
    # wp[row, p*1024 + kc*128 + f] = w[kc*128+row, p*128+f]
    t = np.asarray(w).reshape(8, 128, 2, 128).transpose(1, 2, 0, 3).reshape(128, 2048)
    return np.ascontiguousarray(t).astype(bf)


def _prep_in_maps(x, Wq, Wk, Wv, Wo):
    x = np.asarray(x, dtype=np.float32)
    bf = ml_dtypes.bfloat16
    Wq = np.asarray(Wq, dtype=np.float32)
    Wk = np.asarray(Wk, dtype=np.float32)
    Wv = np.asarray(Wv, dtype=np.float32)
    Wo = np.asarray(Wo, dtype=np.float32)
    ones_b = np.ones((128, 64), dtype=bf)
    tri = (np.arange(128)[None, :] >= np.arange(128)[:, None]).astype(bf)
    xmk = np.zeros((128, 4 * 512), dtype=np.float32)
    for r in range(4):
        for c in range(512):
            d = c - 128 * r
            if d < 0:
                xmk[0, r * 512 + c] = -1024.0
            elif d <= 126:
                xmk[d + 1, r * 512 + c] = -1024.0
    xmk = xmk.astype(bf)
    in_maps = []
    for c in range(8):
        b, g = divmod(c, 4)
        sl = slice(g * FPC, (g + 1) * FPC)
        in_maps.append(
            {
                "xt": np.ascontiguousarray(x[b].T).astype(bf),
                "wq_p": _prepack_pair(Wq[sl, :].T, bf),
                "wk_p": _prepack_pair(Wk[sl, :].T, bf),
                "wv_t": _prepack(Wv[sl, :].T, bf),
                "wo_t": _prepack(Wo[:, sl].T, bf),
                "ones_b": ones_b,
                "tri": tri,
                "xmk": xmk,
            }
        )
    return in_maps


def _get_nc():
    if "nc" not in _CACHE:
        _CACHE["nc"] = _build()
    return _CACHE["nc"]


def _assemble(results):
    out = np.empty((B, T, D), dtype=np.float32)
    for b in range(B):
        out[b] = (
            results[4 * b]["po"].astype(np.float32)
            + results[4 * b + 1]["po"].astype(np.float32)
            + results[4 * b + 2]["po"].astype(np.float32)
            + results[4 * b + 3]["po"].astype(np.float32)
        )
    return out


def kernel(x, Wq, Wk, Wv, Wo):
    nc = _get_nc()
    in_maps = _prep_in_maps(x, Wq, Wk, Wv, Wo)
    res = run_bass_kernel_spmd(nc, in_maps, core_ids=list(range(8)))
    return _assemble(res.results)


def kernel_with_trace(x, Wq, Wk, Wv, Wo, **kw):
    nc = _get_nc()
    in_maps = _prep_in_maps(x, Wq, Wk, Wv, Wo)
    res = run_bass_kernel_spmd(nc, in_maps, core_ids=list(range(8)), trace=True, **kw)
    return _assemble(res.results), res


# revision 8
# speedup vs baseline: 1.2732x; 1.0506x over previous
"""Multi-head causal self-attention (B=2, T=2048, D=1024, H=16) on 8 trn2 cores.

Sharding: data-parallel over batch (cores 0-3 -> batch 0, 4-7 -> batch 1),
tensor-parallel over heads within each 4-core group (4 heads per core).
Wq/Wk/Wv column-sharded, Wo row-sharded; each core emits its partial output
projection and the host sums the 4 partials per batch (TP unshard).

v3 pipeline, engine budget driven (per core, warm):
  ACT (exp): ~82us with diagonal-tile column skip  -- critical resource
  PE: ~90us (scores quad-pairs ~385ns, attV FWL-chained, proj chains 109ns/MM)
  DVE: ~78us (proj casts, causal masks in-place on exp tiles, normalize)
Schedule: projections dripped into the attention steady state; attV pended
2 steps behind exp; normalize DMAs issued from the (idle) GpSimd queue to
avoid Sync head-of-line blocking; PE warmed with dummy matmuls during the
input DMAs.  PSUM: scores 2x2 + oA/oB 2 + proj 2 = 8 banks.
"""

import sys
from collections import deque

for _p in ("/opt/trn_rl_repo", "/root/.axon_site/_ro/trn_rl_repo"):
    if _p not in sys.path:
        sys.path.append(_p)

import ml_dtypes
import numpy as np

import concourse.bass as bass
import concourse.mybir as mybir
import concourse.tile as tile
from concourse import bacc
from concourse.bass_utils import run_bass_kernel_spmd

F32 = mybir.dt.float32
BF16 = mybir.dt.bfloat16

B, T, D = 2, 2048, 1024
H, DH = 16, 64
HPC = 4          # heads per core
FPC = HPC * DH   # feature dims per core (256)
NKT = T // 128   # 16 key tiles / token tiles
NQB = T // 512   # 4 query blocks
VW = 128         # v slot width: 64 dims + ones col 64 + zero pad (FWL needs 128)

DIAG_SKIP = True
N_WARM_MM = 14

_CACHE = {}


def _build():
    nc = bacc.Bacc("TRN2", target_bir_lowering=False, debug=False, num_devices=8)

    xt_d = nc.dram_tensor("xt", [D, T], BF16, kind="ExternalInput").ap()
    wq_d = nc.dram_tensor("wq_p", [128, 2 * D], BF16, kind="ExternalInput").ap()
    wk_d = nc.dram_tensor("wk_p", [128, 2 * D], BF16, kind="ExternalInput").ap()
    wv_d = nc.dram_tensor("wv_t", [128, 8 * FPC], BF16, kind="ExternalInput").ap()
    wo_d = nc.dram_tensor("wo_t", [128, 2 * D], BF16, kind="ExternalInput").ap()
    onesb_d = nc.dram_tensor("ones_b", [128, 64], BF16, kind="ExternalInput").ap()
    masks_d = nc.dram_tensor("masks", [128, 4 * 1024], BF16, kind="ExternalInput").ap()
    out_d = nc.dram_tensor("po", [T, D], BF16, kind="ExternalOutput").ap()
    rscr_d = nc.dram_tensor("rscr", [8, 1024], F32).ap()

    with tile.TileContext(nc) as tc:
        with (
            tc.tile_pool(name="wp", bufs=1) as wp,
            tc.tile_pool(name="xp", bufs=1) as xp,
            tc.tile_pool(name="qk", bufs=1) as qk,
            tc.tile_pool(name="vp", bufs=1) as vp,
            tc.tile_pool(name="at", bufs=1) as at,
            tc.tile_pool(name="ep", bufs=1) as ep,
            tc.tile_pool(name="ob", bufs=1) as ob,
            tc.tile_pool(name="nr", bufs=2) as nr,
            tc.tile_pool(name="op", bufs=3) as op,
            tc.tile_pool(name="pss", bufs=1, space="PSUM") as pss,
            tc.tile_pool(name="pso", bufs=1, space="PSUM") as pso,
            tc.tile_pool(name="psp", bufs=1, space="PSUM") as psp,
        ):
            # ---- persistent SBUF tiles ----
            wq_sb = wp.tile([128, 2 * D], BF16)
            wk_sb = wp.tile([128, 2 * D], BF16)
            wv_sb = wp.tile([128, 8 * FPC], BF16)
            wo_sb = wp.tile([128, 2 * D], BF16)
            masks_sb = wp.tile([128, 4 * 1024], BF16)
            xT = xp.tile([128, 8 * T], BF16)          # chunk kc at cols kc*T
            qT_sb = qk.tile([128, 2 * T], BF16)       # pair hp at cols hp*T
            kT_sb = qk.tile([128, 2 * T], BF16)
            v_sb = vp.tile([128, NKT * HPC * VW], BF16)
            attT_sb = at.tile([128, 2 * T], BF16)
            scr_sb = wp.tile([128, 512], BF16)        # warmup matmul operands

            # warm the ACT exp table during the input DMAs
            warm_a = nr.tile([1, 8], F32, tag="warm", bufs=1)
            warm_b = nr.tile([1, 8], F32, tag="warm2", bufs=1)
            nc.vector.memset(warm_a[:], 0.0)
            nc.scalar.activation(warm_b[:], warm_a[:], mybir.ActivationFunctionType.Exp)
            nc.vector.memset(scr_sb[:], 0.0)
            # zero v pad cols (64:128 of each slot; col 64 overwritten by ones)
            nc.vector.memset(
                v_sb[:].rearrange("p (a b) -> p a b", b=VW)[:, :, 64:128], 0.0
            )

            # ---- input DMAs, priority order ----
            nc.sync.dma_start(wq_sb[:, 0:D], wq_d[:, 0:D])
            nc.sync.dma_start(wk_sb[:, 0:D], wk_d[:, 0:D])
            nc.sync.dma_start(
                xT[:].rearrange("p (c t) -> p c t", t=T)[:, :, 0:512],
                xt_d.rearrange("(c p) t -> p c t", p=128)[:, :, 0:512],
            )
            nc.sync.dma_start(masks_sb[:], masks_d)
            nc.sync.dma_start(wv_sb[:], wv_d)
            nc.sync.dma_start(
                v_sb[:].rearrange("p (a b) -> p a b", b=VW)[:, :, 64],
                onesb_d[:, 0 : NKT * HPC],
            )
            nc.sync.dma_start(wq_sb[:, D : 2 * D], wq_d[:, D : 2 * D])
            nc.sync.dma_start(wk_sb[:, D : 2 * D], wk_d[:, D : 2 * D])
            nc.sync.dma_start(
                xT[:].rearrange("p (c t) -> p c t", t=T)[:, :, 512:T],
                xt_d.rearrange("(c p) t -> p c t", p=128)[:, :, 512:T],
            )
            nc.sync.dma_start(wo_sb[:], wo_d)

            # ---- PE warmup: dummy matmuls on scratch while inputs stream ----
            wps = psp.tile([128, 512], F32, tag="proj", bufs=2, name="warm_ps")
            for i in range(N_WARM_MM):
                nc.tensor.matmul(
                    wps[:], scr_sb[:, 0:128], scr_sb[:],
                    start=(i == 0), stop=(i == N_WARM_MM - 1),
                )

            # ---- emission helpers ----
            emitted = set()
            backlog = deque()
            pend = deque()

            def emit_qk(hp, tb):
                for w_sb, dst in ((wq_sb, qT_sb), (wk_sb, kT_sb)):
                    ps = psp.tile([128, 512], F32, tag="proj", bufs=2, name="qk_ps")
                    for kc in range(8):
                        nc.tensor.matmul(
                            ps[:],
                            w_sb[:, hp * D + kc * 128 : hp * D + (kc + 1) * 128],
                            xT[:, kc * T + tb * 512 : kc * T + (tb + 1) * 512],
                            start=(kc == 0), stop=(kc == 7),
                        )
                    nc.vector.tensor_copy(
                        dst[:, hp * T + tb * 512 : hp * T + (tb + 1) * 512], ps[:]
                    )

            def do_qk(hp, tb):
                key = ("qk", hp, tb)
                if key in emitted:
                    return
                emitted.add(key)
                emit_qk(hp, tb)

            def emit_v(tt):
                ps = psp.tile([128, 512], F32, tag="proj", bufs=2, name="v_ps")
                for kc in range(8):
                    nc.tensor.matmul(
                        ps[:, 0:FPC],
                        xT[:, kc * T + tt * 128 : kc * T + (tt + 1) * 128],
                        wv_sb[:, kc * FPC : (kc + 1) * FPC],
                        start=(kc == 0), stop=(kc == 7),
                    )
                nc.vector.tensor_copy(
                    v_sb[:].rearrange("p (a b) -> p a b", b=VW)[
                        :, tt * HPC : (tt + 1) * HPC, 0:DH
                    ],
                    ps[:, 0:FPC].rearrange("p (a b) -> p a b", b=DH),
                )

            def do_v(tt):
                key = ("v", tt)
                if key in emitted:
                    return
                emitted.add(key)
                emit_v(tt)

            def emit_outproj_tt(qb, t4):
                tt = qb * 4 + t4
                o_sb = op.tile([128, D], BF16, tag="osb", name="o_sb")
                wo_ps = [
                    psp.tile([128, 512], F32, tag="proj", bufs=2, name="wo_ps")
                    for _ in range(2)
                ]
                for hp in range(2):
                    for nck in range(2):
                        nc.tensor.matmul(
                            wo_ps[nck][:],
                            attT_sb[:, hp * T + tt * 128 : hp * T + (tt + 1) * 128],
                            wo_sb[:, hp * D + nck * 512 : hp * D + (nck + 1) * 512],
                            start=(hp == 0), stop=(hp == 1),
                        )
                for nck in range(2):
                    nc.vector.tensor_copy(
                        o_sb[:, nck * 512 : (nck + 1) * 512], wo_ps[nck][:]
                    )
                nc.sync.dma_start(out_d[tt * 128 : (tt + 1) * 128, :], o_sb[:])

            def emit_scores(qb, hp, kt):
                sAB = pss.tile([128, 1024], F32, tag="sAB", bufs=2, name="sAB")
                r = kt - 4 * qb
                qs = 128 * r if (r > 0 and DIAG_SKIP) else 0
                for h, tp in ((0, (0, 0)), (1, (64, 0))):
                    nc.tensor.matmul(
                        sAB[:, h * 512 + qs : (h + 1) * 512],
                        kT_sb[h * 64 : (h + 1) * 64, hp * T + kt * 128 : hp * T + (kt + 1) * 128],
                        qT_sb[h * 64 : (h + 1) * 64, hp * T + qb * 512 + qs : hp * T + (qb + 1) * 512],
                        start=True, stop=True, tile_position=tp,
                    )
                return sAB, qs

            def emit_act_mask(qb, hp, kt, sAB, qs):
                r = kt - 4 * qb
                eAB = ep.tile([128, 1024], BF16, tag="eAB", bufs=4, name="eAB")
                if qs == 0:
                    nc.scalar.activation(
                        eAB[:], sAB[:], mybir.ActivationFunctionType.Exp, scale=0.125
                    )
                    if r >= 0:
                        nc.vector.tensor_mul(
                            eAB[:], eAB[:], masks_sb[:, r * 1024 : (r + 1) * 1024]
                        )
                else:
                    iv = sAB[:].rearrange("p (h q) -> p h q", h=2)[:, :, qs:512]
                    ov = eAB[:].rearrange("p (h q) -> p h q", h=2)[:, :, qs:512]
                    nc.scalar.activation(
                        ov, iv, mybir.ActivationFunctionType.Exp, scale=0.125
                    )
                    mv = masks_sb[:, r * 1024 : (r + 1) * 1024].rearrange(
                        "p (h q) -> p h q", h=2
                    )[:, :, qs:512]
                    nc.vector.tensor_mul(ov, ov, mv)
                return eAB

            def emit_attv(qb, hp, kt, eAB, qs, oA, oB):
                nkt = 4 * (qb + 1)
                for h, o_ps in ((0, oA), (1, oB)):
                    nc.tensor.matmul(
                        o_ps[:, qs:512],
                        v_sb[:, (kt * HPC + 2 * hp + h) * VW : (kt * HPC + 2 * hp + h + 1) * VW],
                        eAB[:, h * 512 + qs : (h + 1) * 512],
                        start=(kt == 0), stop=(kt == nkt - 1),
                    )

            def emit_norm(ui, qb, hp, oA, oB):
                oAs = ob.tile([VW, 512], F32, tag="oAs", bufs=2, name="oAs")
                oBs = ob.tile([VW, 512], F32, tag="oBs", bufs=2, name="oBs")
                nc.vector.tensor_copy(oAs[0:65, :], oA[0:65, :])
                nc.vector.tensor_copy(oBs[0:65, :], oB[0:65, :])
                packed = nr.tile([128, 8], F32, tag="packed", name="packed")
                nc.gpsimd.dma_start(
                    packed[:, 0:4], oAs[64:65, :].rearrange("r (g e) -> r g e", e=4)
                )
                nc.gpsimd.dma_start(
                    packed[:, 4:8], oBs[64:65, :].rearrange("r (g e) -> r g e", e=4)
                )
                rpacked = nr.tile([128, 8], F32, tag="rpacked", name="rpacked")
                nc.vector.reciprocal(rpacked[:], packed[:])
                rrow_d = rscr_d[ui : ui + 1, :]
                nc.gpsimd.dma_start(
                    rrow_d[:, 0:512].rearrange("r (g e) -> r g e", e=4), rpacked[:, 0:4]
                )
                nc.gpsimd.dma_start(
                    rrow_d[:, 512:1024].rearrange("r (g e) -> r g e", e=4),
                    rpacked[:, 4:8],
                )
                bcs = []
                for off in (0, 512):
                    bc = nr.tile([64, 512], F32, tag="bc", name="bc")
                    nc.gpsimd.dma_start(
                        bc[:], rrow_d[0:1, off : off + 512].partition_broadcast(64)
                    )
                    bcs.append(bc)

                def muls(oAs=oAs, oBs=oBs, bcs=bcs, qb=qb, hp=hp):
                    for o_s, bc, prow in ((oAs, bcs[0], 0), (oBs, bcs[1], 64)):
                        nc.vector.tensor_mul(
                            attT_sb[prow : prow + 64, hp * T + qb * 512 : hp * T + (qb + 1) * 512],
                            o_s[0:64, :],
                            bc[:],
                        )
                backlog.append(muls)
                if hp == 1:
                    for t4 in range(4):
                        backlog.append(lambda qb=qb, t4=t4: emit_outproj_tt(qb, t4))

            def pop_attv():
                qb_, hp_, kt_, ui_, eAB_, qs_, oA_, oB_ = pend.popleft()
                do_v(kt_)
                emit_attv(qb_, hp_, kt_, eAB_, qs_, oA_, oB_)
                if kt_ == 4 * (qb_ + 1) - 1:
                    emit_norm(ui_, qb_, hp_, oA_, oB_)

            def drip(n):
                for _ in range(n):
                    if backlog:
                        backlog.popleft()()

            unit_pushes = {
                0: [lambda: do_qk(1, 0)],
                1: [lambda: do_qk(0, 1)] + [
                    (lambda tt=tt: do_v(tt)) for tt in range(4, 8)
                ],
                2: [lambda: do_qk(1, 1)],
                3: [lambda: do_qk(0, 2)] + [
                    (lambda tt=tt: do_v(tt)) for tt in range(8, 12)
                ],
                4: [lambda: do_qk(1, 2)],
                5: [lambda: do_qk(0, 3)] + [
                    (lambda tt=tt: do_v(tt)) for tt in range(12, 16)
                ],
                6: [lambda: do_qk(1, 3)],
                7: [],
            }

            # ---- main pipeline ----
            do_qk(0, 0)
            ui = 0
            for qb in range(NQB):
                for hp in range(2):
                    do_qk(hp, qb)
                    backlog.extend(unit_pushes[ui])
                    oA = pso.tile([VW, 512], F32, tag="oA", bufs=1, name="oA")
                    oB = pso.tile([VW, 512], F32, tag="oB", bufs=1, name="oB")
                    nkt = 4 * (qb + 1)
                    for kt in range(nkt):
                        do_v(kt)  # ensure v ready 2 steps before its attv
                        sAB, qs = emit_scores(qb, hp, kt)
                        eAB = emit_act_mask(qb, hp, kt, sAB, qs)
                        pend.append((qb, hp, kt, ui, eAB, qs, oA, oB))
                        if len(pend) > 2:
                            pop_attv()
                        steps_left = nkt - kt
                        n = 1 if backlog else 0
                        if len(backlog) > steps_left:
                            n = 2
                        drip(n)
                    ui += 1
            while pend:
                pop_attv()
            drip(len(backlog))

    nc.compile()
    return nc


def _prepack(w, bf):
    # [c*128, f] -> [128, c*f] (SBUF chunk layout)
    c = w.shape[0] // 128
    return np.ascontiguousarray(
        w.reshape(c, 128, w.shape[1]).transpose(1, 0, 2).reshape(128, -1)
    ).astype(bf)


def _prepack_pair(w, bf):
    # w: [1024, 256] (d_model x pair features) -> [128, 2*1024] pair-major:
    # wp[row, p*1024 + kc*128 + f] = w[kc*128+row, p*128+f]
    t = np.asarray(w).reshape(8, 128, 2, 128).transpose(1, 2, 0, 3).reshape(128, 2048)
    return np.ascontiguousarray(t).astype(bf)


def _prep_in_maps(x, Wq, Wk, Wv, Wo):
    x = np.asarray(x, dtype=np.float32)
    bf = ml_dtypes.bfloat16
    Wq = np.asarray(Wq, dtype=np.float32)
    Wk = np.asarray(Wk, dtype=np.float32)
    Wv = np.asarray(Wv, dtype=np.float32)
    Wo = np.asarray(Wo, dtype=np.float32)
    ones_b = np.ones((128, 64), dtype=bf)
    ii = np.arange(128)[:, None]
    qq = np.arange(512)[None, :]
    masks = np.concatenate(
        [np.tile((qq >= ii + 128 * r).astype(bf), (1, 2)) for r in range(4)],
        axis=1,
    )
    in_maps = []
    for c in range(8):
        b, g = divmod(c, 4)
        sl = slice(g * FPC, (g + 1) * FPC)
        in_maps.append(
            {
                "xt": np.ascontiguousarray(x[b].T).astype(bf),
                "wq_p": _prepack_pair(Wq[sl, :].T, bf),
                "wk_p": _prepack_pair(Wk[sl, :].T, bf),
                "wv_t": _prepack(Wv[sl, :].T, bf),
                "wo_t": _prepack(Wo[:, sl].T, bf),
                "ones_b": ones_b,
                "masks": masks,
            }
        )
    return in_maps


def _get_nc():
    if "nc" not in _CACHE:
        _CACHE["nc"] = _build()
    return _CACHE["nc"]


def _assemble(results):
    out = np.empty((B, T, D), dtype=np.float32)
    for b in range(B):
        out[b] = (
            results[4 * b]["po"].astype(np.float32)
            + results[4 * b + 1]["po"].astype(np.float32)
            + results[4 * b + 2]["po"].astype(np.float32)
            + results[4 * b + 3]["po"].astype(np.float32)
        )
    return out


def kernel(x, Wq, Wk, Wv, Wo):
    nc = _get_nc()
    in_maps = _prep_in_maps(x, Wq, Wk, Wv, Wo)
    res = run_bass_kernel_spmd(nc, in_maps, core_ids=list(range(8)))
    return _assemble(res.results)


def kernel_with_trace(x, Wq, Wk, Wv, Wo, **kw):
    nc = _get_nc()
    in_maps = _prep_in_maps(x, Wq, Wk, Wv, Wo)
    res = run_bass_kernel_spmd(nc, in_maps, core_ids=list(range(8)), trace=True, **kw)
    return _assemble(res.results), res


# revision 40
# speedup vs baseline: 1.2933x; 1.0158x over previous
"""Multi-head causal self-attention (B=2, T=2048, D=1024, H=16) on 8 trn2 cores.

Sharding: data-parallel over batch (cores 0-3 -> batch 0, 4-7 -> batch 1),
tensor-parallel over heads within each 4-core group (4 heads per core).
Wq/Wk/Wv column-sharded, Wo row-sharded; each core emits its partial output
projection and the host sums the 4 partials per batch (TP unshard).

v3 pipeline, engine budget driven (per core, warm):
  ACT (exp): ~82us with diagonal-tile column skip  -- critical resource
  PE: ~90us (scores quad-pairs ~385ns, attV FWL-chained, proj chains 109ns/MM)
  DVE: ~78us (proj casts, causal masks in-place on exp tiles, normalize)
Schedule: projections dripped into the attention steady state; attV pended
2 steps behind exp; normalize DMAs issued from the (idle) GpSimd queue to
avoid Sync head-of-line blocking; PE warmed with dummy matmuls during the
input DMAs.  PSUM: scores 2x2 + oA/oB 2 + proj 2 = 8 banks.
"""

import sys
from collections import deque

for _p in ("/opt/trn_rl_repo", "/root/.axon_site/_ro/trn_rl_repo"):
    if _p not in sys.path:
        sys.path.append(_p)

import ml_dtypes
import numpy as np

import concourse.bass as bass
import concourse.mybir as mybir
import concourse.tile as tile
from concourse import bacc
from concourse.bass_utils import run_bass_kernel_spmd

F32 = mybir.dt.float32
BF16 = mybir.dt.bfloat16

B, T, D = 2, 2048, 1024
H, DH = 16, 64
HPC = 4          # heads per core
FPC = HPC * DH   # feature dims per core (256)
NKT = T // 128   # 16 key tiles / token tiles
NQB = T // 512   # 4 query blocks
VW = 128         # v slot width: 64 dims + ones col 64 + zero pad (FWL needs 128)

DIAG_SKIP = True
N_WARM_MM = 8

_CACHE = {}


def _build():
    nc = bacc.Bacc("TRN2", target_bir_lowering=False, debug=False, num_devices=8)

    xt_d = nc.dram_tensor("xt", [D, T], BF16, kind="ExternalInput").ap()
    wq_d = nc.dram_tensor("wq_p", [128, 2 * D], BF16, kind="ExternalInput").ap()
    wk_d = nc.dram_tensor("wk_p", [128, 2 * D], BF16, kind="ExternalInput").ap()
    wv_d = nc.dram_tensor("wv_t", [128, 8 * FPC], BF16, kind="ExternalInput").ap()
    wo_d = nc.dram_tensor("wo_t", [128, 2 * D], BF16, kind="ExternalInput").ap()
    masks_d = nc.dram_tensor("masks", [128, 4 * 1024], BF16, kind="ExternalInput").ap()
    out_d = nc.dram_tensor("po", [T, D], BF16, kind="ExternalOutput").ap()

    with tile.TileContext(nc) as tc:
        with (
            tc.tile_pool(name="wp", bufs=1) as wp,
            tc.tile_pool(name="xp", bufs=1) as xp,
            tc.tile_pool(name="qk", bufs=1) as qk,
            tc.tile_pool(name="vp", bufs=1) as vp,
            tc.tile_pool(name="at", bufs=1) as at,
            tc.tile_pool(name="ep", bufs=1) as ep,
            tc.tile_pool(name="ob", bufs=1) as ob,
            tc.tile_pool(name="nr", bufs=2) as nr,
            tc.tile_pool(name="op", bufs=3) as op,
            tc.tile_pool(name="pss", bufs=1, space="PSUM") as pss,
            tc.tile_pool(name="pso", bufs=1, space="PSUM") as pso,
            tc.tile_pool(name="psp", bufs=1, space="PSUM") as psp,
        ):
            # ---- persistent SBUF tiles ----
            wq_sb = wp.tile([128, 2 * D], BF16)
            wk_sb = wp.tile([128, 2 * D], BF16)
            wv_sb = wp.tile([128, 8 * FPC], BF16)
            wo_sb = wp.tile([128, 2 * D], BF16)
            masks_sb = wp.tile([128, 4 * 1024], BF16)
            xT = xp.tile([128, 8 * T], BF16)   # tb-major: tb*4096 + kc*512 + t
            qT_sb = qk.tile([128, 2 * T], BF16)       # pair hp at cols hp*T
            kT_sb = qk.tile([128, 2 * T], BF16)
            v_sb = vp.tile([128, NKT * HPC * VW], BF16)
            attT_sb = at.tile([128, 2 * T], BF16)
            scr_sb = wp.tile([128, 512], BF16)        # warmup matmul operands

            # warm the ACT exp table during the input DMAs
            warm_a = nr.tile([1, 8], F32, tag="warm", bufs=1)
            warm_b = nr.tile([1, 8], F32, tag="warm2", bufs=1)
            nc.vector.memset(warm_a[:], 0.0)
            nc.scalar.activation(warm_b[:], warm_a[:], mybir.ActivationFunctionType.Exp)
            nc.vector.memset(scr_sb[:], 0.0)
            # force the gpsimd IRAM library load early (off the critical path)
            warm_g = nr.tile([2, 8], F32, tag="warmg", bufs=1)
            nc.gpsimd.partition_broadcast(warm_g[:], warm_a[0:1, :], channels=2)
            # v slot layout: col 0 = ones (denominator lands on psum partition
            # 0 for partition_broadcast), cols 1:64 zero pad, cols 64:128 =
            # v dims (aligned partition base; 128-wide stationary for FWL)
            nc.gpsimd.memset(
                v_sb[:].rearrange("p (a b) -> p a b", b=VW)[:, :, 0], 1.0
            )
            nc.gpsimd.memset(
                v_sb[:].rearrange("p (a b) -> p a b", b=VW)[:, :, 1:64], 0.0
            )

            # ---- input DMAs, priority order ----
            nc.sync.dma_start(
                xT[:, 0:4096].rearrange("p (c t) -> p c t", t=512),
                xt_d.rearrange("(c p) t -> p c t", p=128)[:, :, 0:512],
            )
            nc.sync.dma_start(wq_sb[:, 0:D], wq_d[:, 0:D])
            nc.sync.dma_start(wk_sb[:, 0:D], wk_d[:, 0:D])
            nc.sync.dma_start(wq_sb[:, D : 2 * D], wq_d[:, D : 2 * D])
            nc.sync.dma_start(wk_sb[:, D : 2 * D], wk_d[:, D : 2 * D])
            nc.sync.dma_start(masks_sb[:], masks_d)
            nc.sync.dma_start(wv_sb[:], wv_d)
            for tb in range(1, 4):
                nc.sync.dma_start(
                    xT[:, tb * 4096 : (tb + 1) * 4096].rearrange("p (c t) -> p c t", t=512),
                    xt_d.rearrange("(c p) t -> p c t", p=128)[:, :, tb * 512 : (tb + 1) * 512],
                )
            nc.sync.dma_start(wo_sb[:], wo_d)

            # ---- PE warmup: dummy matmuls on scratch while inputs stream ----
            wps = psp.tile([128, 512], F32, tag="proj", bufs=2, name="warm_ps")
            for i in range(N_WARM_MM):
                nc.tensor.matmul(
                    wps[:, 0:256], scr_sb[:, 0:128], scr_sb[:, 0:256],
                    start=(i == 0), stop=(i == N_WARM_MM - 1),
                )

            # ---- emission helpers ----
            emitted = set()
            backlog = deque()
            pend = deque()
            scalar_casts = [0]  # early proj casts routed to the idle ACT engine

            def proj_cast(dst, src):
                if scalar_casts[0] > 0:
                    scalar_casts[0] -= 1
                    nc.scalar.copy(dst, src)
                else:
                    nc.vector.tensor_copy(dst, src)

            def emit_qk_half(hp, tb, half):
                w_sb, dst = ((wq_sb, qT_sb), (wk_sb, kT_sb))[half]
                ps = psp.tile([128, 512], F32, tag="proj", bufs=2, name="qk_ps")
                for kc in range(8):
                    nc.tensor.matmul(
                        ps[:],
                        w_sb[:, hp * D + kc * 128 : hp * D + (kc + 1) * 128],
                        xT[:, tb * 4096 + kc * 512 : tb * 4096 + (kc + 1) * 512],
                        start=(kc == 0), stop=(kc == 7),
                    )
                proj_cast(dst[:, hp * T + tb * 512 : hp * T + (tb + 1) * 512], ps[:])

            def do_qk_half(hp, tb, half):
                key = ("qk", hp, tb, half)
                if key in emitted:
                    return
                emitted.add(key)
                emit_qk_half(hp, tb, half)

            def do_qk(hp, tb):
                do_qk_half(hp, tb, 0)
                do_qk_half(hp, tb, 1)

            def emit_v(tt):
                ps = psp.tile([128, 512], F32, tag="proj", bufs=2, name="v_ps")
                for kc in range(8):
                    nc.tensor.matmul(
                        ps[:, 0:FPC],
                        xT[:, (tt // 4) * 4096 + kc * 512 + (tt % 4) * 128 : (tt // 4) * 4096 + kc * 512 + (tt % 4) * 128 + 128],
                        wv_sb[:, kc * FPC : (kc + 1) * FPC],
                        start=(kc == 0), stop=(kc == 7),
                    )
                proj_cast(
                    v_sb[:].rearrange("p (a b) -> p a b", b=VW)[
                        :, tt * HPC : (tt + 1) * HPC, 64:128
                    ],
                    ps[:, 0:FPC].rearrange("p (a b) -> p a b", b=DH),
                )

            def do_v(tt):
                key = ("v", tt)
                if key in emitted:
                    return
                emitted.add(key)
                emit_v(tt)

            o_sb_map = {}

            def emit_outproj_half(qb, t4, nck):
                tt = qb * 4 + t4
                if nck == 0:
                    o_sb_map[tt] = op.tile([128, D], BF16, tag="osb", name="o_sb")
                o_sb = o_sb_map[tt]
                wo_ps = psp.tile([128, 512], F32, tag="proj", bufs=2, name="wo_ps")
                for hp in range(2):
                    nc.tensor.matmul(
                        wo_ps[:],
                        attT_sb[:, hp * T + tt * 128 : hp * T + (tt + 1) * 128],
                        wo_sb[:, hp * D + nck * 512 : hp * D + (nck + 1) * 512],
                        start=(hp == 0), stop=(hp == 1),
                    )
                if qb == 3 and nck == 0:
                    # tail: exp stream is done, use the idle scalar engine
                    nc.scalar.copy(o_sb[:, 0:512], wo_ps[:])
                else:
                    nc.vector.tensor_copy(o_sb[:, nck * 512 : (nck + 1) * 512], wo_ps[:])
                if nck == 1:
                    nc.gpsimd.dma_start(out_d[tt * 128 : (tt + 1) * 128, :], o_sb[:])

            def emit_scores(qb, hp, kt):
                sAB = pss.tile([128, 1024], F32, tag="sAB", bufs=2, name="sAB")
                r = kt - 4 * qb
                qs = 128 * r if (r > 0 and DIAG_SKIP) else 0
                for h, tp in ((0, (0, 0)), (1, (64, 0))):
                    nc.tensor.matmul(
                        sAB[:, h * 512 + qs : (h + 1) * 512],
                        kT_sb[h * 64 : (h + 1) * 64, hp * T + kt * 128 : hp * T + (kt + 1) * 128],
                        qT_sb[h * 64 : (h + 1) * 64, hp * T + qb * 512 + qs : hp * T + (qb + 1) * 512],
                        start=True, stop=True, tile_position=tp,
                    )
                return sAB, qs

            def emit_act_mask(qb, hp, kt, sAB, qs):
                r = kt - 4 * qb
                eAB = ep.tile([128, 1024], BF16, tag="eAB", bufs=6, name="eAB")
                if qs == 0:
                    nc.scalar.activation(
                        eAB[:], sAB[:], mybir.ActivationFunctionType.Exp, scale=0.125
                    )
                    if r >= 0:
                        nc.vector.tensor_mul(
                            eAB[:], eAB[:], masks_sb[:, r * 1024 : (r + 1) * 1024]
                        )
                else:
                    iv = sAB[:].rearrange("p (h q) -> p h q", h=2)[:, :, qs:512]
                    ov = eAB[:].rearrange("p (h q) -> p h q", h=2)[:, :, qs:512]
                    nc.scalar.activation(
                        ov, iv, mybir.ActivationFunctionType.Exp, scale=0.125
                    )
                    mv = masks_sb[:, r * 1024 : (r + 1) * 1024].rearrange(
                        "p (h q) -> p h q", h=2
                    )[:, :, qs:512]
                    nc.vector.tensor_mul(ov, ov, mv)
                return eAB

            def emit_attv(qb, hp, kt, eAB, qs, oA, oB):
                nkt = 4 * (qb + 1)
                for h, o_ps in ((0, oA), (1, oB)):
                    nc.tensor.matmul(
                        o_ps[:, qs:512],
                        v_sb[:, (kt * HPC + 2 * hp + h) * VW : (kt * HPC + 2 * hp + h + 1) * VW],
                        eAB[:, h * 512 + qs : (h + 1) * 512],
                        start=(kt == 0), stop=(kt == nkt - 1),
                    )

            def emit_norm(ui, qb, hp, oA, oB):
                oAs = ob.tile([128, 512], F32, tag="oAs", bufs=2, name="oAs")
                oBs = ob.tile([128, 512], F32, tag="oBs", bufs=2, name="oBs")
                nc.vector.tensor_copy(oAs[:], oA[:])
                nc.vector.tensor_copy(oBs[:], oB[:])
                if ui == 7:
                    # tail: latency matters, DVE is idle -- broadcast the raw
                    # denominators and take the big reciprocal directly
                    bcs = []
                    for o_s in (oAs, oBs):
                        bc = nr.tile([128, 512], F32, tag="bc", bufs=4, name="bc")
                        nc.gpsimd.partition_broadcast(bc[:], o_s[0:1, :], channels=128)
                        nc.vector.reciprocal(bc[64:128, :], bc[64:128, :])
                        bcs.append(bc)

                    def tmuls(oAs=oAs, oBs=oBs, bcs=bcs, qb=qb, hp=hp):
                        for o_s, bc, prow in ((oAs, bcs[0], 0), (oBs, bcs[1], 64)):
                            nc.vector.tensor_mul(
                                attT_sb[prow : prow + 64, hp * T + qb * 512 : hp * T + (qb + 1) * 512],
                                o_s[64:128, :],
                                bc[64:128, :],
                            )
                    backlog.append(tmuls)
                    if hp == 1:
                        for t4 in range(4):
                            for nck in range(2):
                                backlog.append(
                                    lambda qb=qb, t4=t4, nck=nck: emit_outproj_half(qb, t4, nck)
                                )
                    return
                packed = nr.tile([128, 8], F32, tag="packed", name="packed")
                nc.sync.dma_start(
                    packed[:, 0:4], oAs[0:1, :].rearrange("r (g e) -> r g e", e=4)
                )
                nc.sync.dma_start(
                    packed[:, 4:8], oBs[0:1, :].rearrange("r (g e) -> r g e", e=4)
                )
                rpacked = nr.tile([128, 8], F32, tag="rpacked", name="rpacked")
                nc.vector.reciprocal(rpacked[:], packed[:])
                rrecs = []
                for h in range(2):
                    rrec = nr.tile([1, 512], F32, tag=f"rrec{h}", name="rrec")
                    nc.sync.dma_start(
                        rrec[:].rearrange("r (g e) -> r g e", e=4),
                        rpacked[:, 4 * h : 4 * h + 4],
                    )
                    rrecs.append(rrec)
                bcs = []
                for rrec in rrecs:
                    bc = nr.tile([128, 512], F32, tag="bc", bufs=4, name="bc")
                    nc.gpsimd.partition_broadcast(bc[:], rrec[:], channels=128)
                    bcs.append(bc)

                def muls(oAs=oAs, oBs=oBs, bcs=bcs, qb=qb, hp=hp):
                    for o_s, bc, prow in ((oAs, bcs[0], 0), (oBs, bcs[1], 64)):
                        nc.vector.tensor_mul(
                            attT_sb[prow : prow + 64, hp * T + qb * 512 : hp * T + (qb + 1) * 512],
                            o_s[64:128, :],
                            bc[64:128, :],
                        )
                backlog.append(muls)
                if hp == 1:
                    for t4 in range(4):
                        for nck in range(2):
                            backlog.append(
                                lambda qb=qb, t4=t4, nck=nck: emit_outproj_half(qb, t4, nck)
                            )

            def pop_attv():
                qb_, hp_, kt_, ui_, eAB_, qs_, oA_, oB_ = pend.popleft()
                do_v(kt_)
                emit_attv(qb_, hp_, kt_, eAB_, qs_, oA_, oB_)
                if kt_ == 4 * (qb_ + 1) - 1:
                    emit_norm(ui_, qb_, hp_, oA_, oB_)

            def drip(n):
                for _ in range(n):
                    if backlog:
                        backlog.popleft()()

            def qk_items(hp, tb):
                return [
                    (lambda hp=hp, tb=tb: do_qk_half(hp, tb, 0)),
                    (lambda hp=hp, tb=tb: do_qk_half(hp, tb, 1)),
                ]

            unit_pushes = {
                0: qk_items(1, 0),
                1: qk_items(0, 1) + [
                    (lambda tt=tt: do_v(tt)) for tt in range(4, 8)
                ],
                2: qk_items(1, 1),
                3: qk_items(0, 2) + [
                    (lambda tt=tt: do_v(tt)) for tt in range(8, 12)
                ],
                4: qk_items(1, 2),
                5: qk_items(0, 3) + [
                    (lambda tt=tt: do_v(tt)) for tt in range(12, 16)
                ],
                6: qk_items(1, 3),
                7: [],
            }

            # ---- main pipeline ----
            do_qk(0, 0)
            ui = 0
            for qb in range(NQB):
                for hp in range(2):
                    do_qk(hp, qb)
                    backlog.extend(unit_pushes[ui])
                    oA = pso.tile([128, 512], F32, tag="oA", bufs=1, name="oA")
                    oB = pso.tile([128, 512], F32, tag="oB", bufs=1, name="oB")
                    nkt = 4 * (qb + 1)
                    for kt in range(nkt):
                        sAB, qs = emit_scores(qb, hp, kt)
                        eAB = emit_act_mask(qb, hp, kt, sAB, qs)
                        do_v(kt)  # ensure v ready 2 steps before its attv
                        pend.append((qb, hp, kt, ui, eAB, qs, oA, oB))
                        if len(pend) > 2:
                            pop_attv()
                        drip(1)
                    ui += 1
            while pend:
                pop_attv()
            tw = pso.tile([128, 512], F32, tag="oA", bufs=1, name="tail_warm")
            for i in range(30):
                nc.tensor.matmul(
                    tw[:], scr_sb[:, 0:128], scr_sb[:],
                    start=(i == 0), stop=(i == 29),
                )
            drip(len(backlog))

    nc.compile()
    return nc


def _prepack(w, bf):
    # [c*128, f] -> [128, c*f] (SBUF chunk layout)
    c = w.shape[0] // 128
    return np.ascontiguousarray(
        w.reshape(c, 128, w.shape[1]).transpose(1, 0, 2).reshape(128, -1)
    ).astype(bf)


def _prepack_pair(w, bf):
    # w: [1024, 256] (d_model x pair features) -> [128, 2*1024] pair-major:
    # wp[row, p*1024 + kc*128 + f] = w[kc*128+row, p*128+f]
    t = np.asarray(w).reshape(8, 128, 2, 128).transpose(1, 2, 0, 3).reshape(128, 2048)
    return np.ascontiguousarray(t).astype(bf)


def _prep_in_maps(x, Wq, Wk, Wv, Wo):
    x = np.asarray(x, dtype=np.float32)
    bf = ml_dtypes.bfloat16
    Wq = np.asarray(Wq, dtype=np.float32)
    Wk = np.asarray(Wk, dtype=np.float32)
    Wv = np.asarray(Wv, dtype=np.float32)
    Wo = np.asarray(Wo, dtype=np.float32)
    ii = np.arange(128)[:, None]
    qq = np.arange(512)[None, :]
    masks = np.concatenate(
        [np.tile((qq >= ii + 128 * r).astype(bf), (1, 2)) for r in range(4)],
        axis=1,
    )
    in_maps = []
    for c in range(8):
        b, g = divmod(c, 4)
        sl = slice(g * FPC, (g + 1) * FPC)
        in_maps.append(
            {
                "xt": np.ascontiguousarray(x[b].T).astype(bf),
                "wq_p": _prepack_pair(Wq[sl, :].T, bf),
                "wk_p": _prepack_pair(Wk[sl, :].T, bf),
                "wv_t": _prepack(Wv[sl, :].T, bf),
                "wo_t": _prepack(Wo[:, sl].T, bf),
                "masks": masks,
            }
        )
    return in_maps


def _get_nc():
    if "nc" not in _CACHE:
        _CACHE["nc"] = _build()
    return _CACHE["nc"]


def _assemble(results):
    out = np.empty((B, T, D), dtype=np.float32)
    for b in range(B):
        out[b] = (
            results[4 * b]["po"].astype(np.float32)
            + results[4 * b + 1]["po"].astype(np.float32)
            + results[4 * b + 2]["po"].astype(np.float32)
            + results[4 * b + 3]["po"].astype(np.float32)
        )
    return out


def kernel(x, Wq, Wk, Wv, Wo):
    nc = _get_nc()
    in_maps = _prep_in_maps(x, Wq, Wk, Wv, Wo)
    res = run_bass_kernel_spmd(nc, in_maps, core_ids=list(range(8)))
    return _assemble(res.results)


def kernel_with_trace(x, Wq, Wk, Wv, Wo, **kw):
    nc = _get_nc()
    in_maps = _prep_in_maps(x, Wq, Wk, Wv, Wo)
    res = run_bass_kernel_spmd(nc, in_maps, core_ids=list(range(8)), trace=True, **kw)
    return _assemble(res.results), res
